# revision 20
# baseline (speedup 1.0000x reference)
"""Grouped MLP (8-expert SwiGLU) Trainium2 Bass kernel.

Sharding: expert-parallel, one group per NeuronCore (8 cores).
Token t belongs to group t % 8, so core n gets x[n::8] (4096 tokens),
its expert's gate/up/down weights, and produces out[n::8].

Design (from NTFF trace analysis):
- All matmul operands in bf16 (max rel err ~4e-3 vs the 2e-2 gate):
  halves HBM traffic and SBUF footprint, and enables FWL weight loads
  (LDWEIGHTS 97ns, fully hidden under the 216ns matmul beat).
- Down projection uses wd as the stationary operand, so hidden stays in
  its natural [h, t] layout as the moving operand and out is produced in
  [O, T] layout (host transposes back).
- Dummy warmup matmuls (N=256, no DMA deps) run during the ~7us
  framework preamble + first-DMA-dead window so the PE's HAM clock gate
  is at 8/8 when real matmuls start, and real matmuls are never the ones
  paying the cold-clock penalty.
- Token blocks of 1024 keep the first block's DMA footprint small
  (wg0 + 8 x 256KB xt slices) so the PE reaches steady state ~15us in;
  startup loads are spread across the sync and gpsimd issue queues,
  weight streams ride sync, bulk prefetches and output stores ride
  gpsimd, and the scalar queue carries only silu activations (a DMA
  issued behind activations deadlocks the ramp into data starvation).
- One shared 8-bank PSUM rotation (tag "acc") across gate/up/down.
"""

import sys

if "/opt/trn_rl_repo" not in sys.path:
    sys.path.insert(0, "/opt/trn_rl_repo")

import numpy as np
import ml_dtypes

import concourse.bass as bass  # noqa: F401  (registers bass machinery)
import concourse.tile as tile
from concourse import bacc, mybir
from concourse.bass_utils import run_bass_kernel_spmd

P = 128
T = 4096   # tokens per core (per group)
K = 1024   # d_in
H = 2048   # d_hid
O = 1024   # d_out
N_CORES = 8

F32 = mybir.dt.float32
BF16 = mybir.dt.bfloat16
NPBF = ml_dtypes.bfloat16

# Tiling knobs
TB = 1024           # token block
NTB = T // TB       # 4 token blocks
NT = TB // 512      # 2 moving t-tiles per block
KO = K // P         # 8 k-subtiles
NWC = H // P        # 16 hidden chunks of 128
HO = H // P         # 16 h-subtiles
NOC = O // P        # 8 output column groups
N_WARM = 34         # dummy warmup matmuls at N=256

_CACHED_NC = None


def _build_nc():
    from contextlib import ExitStack

    nc = bacc.Bacc(None, target_bir_lowering=False)
    xt = nc.dram_tensor("xt", [K, T], BF16, kind="ExternalInput")
    wg = nc.dram_tensor("wg", [P, NWC, KO, P], BF16, kind="ExternalInput")
    wu = nc.dram_tensor("wu", [P, NWC, KO, P], BF16, kind="ExternalInput")
    wd = nc.dram_tensor("wd", [P, HO, O], BF16, kind="ExternalInput")
    out = nc.dram_tensor("out", [O, T], F32, kind="ExternalOutput")

    silu_fn = mybir.ActivationFunctionType.Silu

    with tile.TileContext(nc) as tc, ExitStack() as ctx:
        const = ctx.enter_context(tc.tile_pool(name="const", bufs=1))
        xpool = ctx.enter_context(tc.tile_pool(name="xpool", bufs=2))
        wpool = ctx.enter_context(tc.tile_pool(name="wpool", bufs=8))
        hpool = ctx.enter_context(tc.tile_pool(name="hpool", bufs=2))
        spool = ctx.enter_context(tc.tile_pool(name="spool", bufs=4))
        opool = ctx.enter_context(tc.tile_pool(name="opool", bufs=3))
        psum = ctx.enter_context(tc.tile_pool(name="psum", bufs=8, space="PSUM"))

        # PE warmup (HAM clock gate) — no DMA dependencies.  The dummy
        # PSUM tile owns its own bank (tag "warm") so padding matmuls can
        # also be woven between the first data-gated matmuls below without
        # corrupting the accumulator rotation.
        dummy_w = const.tile([P, 512], BF16)
        nc.vector.memset(dummy_w[:], 0)
        dps = psum.tile([P, 512], F32, tag="warm", bufs=1)

        def pad(n):
            for _ in range(n):
                nc.tensor.matmul(
                    dps[:, 0:256], dummy_w[:, 0:P], dummy_w[:, 0:256],
                    start=True, stop=True,
                )

        pad(N_WARM)

        # Down-projection weights: resident for the whole kernel.
        wd_sb = const.tile([P, HO, O], BF16)

        # Variable block sizes: two half blocks first so the startup
        # data cliff is 1.25MB instead of 2.5MB and the PE reaches steady
        # state while the DMA path is still ramping.
        blocks = [(0, 512), (512, 512), (1024, 1024), (2048, 1024), (3072, 1024)]
        NB = len(blocks)
        xt_sbs = [None] * NB
        xt_sbs[0] = xpool.tile([P, KO, blocks[0][1]], BF16, tag="xt", name="xt_sb0")

        for tb, (t0, tbs) in enumerate(blocks):
            nt = tbs // 512
            xt_sb = xt_sbs[tb]
            hid_sb = hpool.tile([P, HO, tbs], BF16, tag="hid")

            for wc in range(NWC):
                wg_sb = wpool.tile([P, KO, P], BF16, tag="wg")
                wu_sb = wpool.tile([P, KO, P], BF16, tag="wu")
                if tb == 0 and wc == 0:
                    # Startup critical path: first weight piece is 32KB so
                    # the first LDWEIGHTS unblocks early; xt slices spread
                    # across all three issue queues (sync / scalar /
                    # gpsimd) so their completion semaphores fire early.
                    # The scalar-queue loads are safe only because they
                    # precede every silu activation in program order.
                    nc.sync.dma_start(wg_sb[:, 0:1], wg[:, 0, 0:1])
                    nc.sync.dma_start(xt_sb[:, 0], xt[0:P, 0:tbs])
                    nc.sync.dma_start(wg_sb[:, 1:8], wg[:, 0, 1:8])
                    nc.sync.dma_start(xt_sb[:, 1], xt[P : 2 * P, 0:tbs])
                    nc.sync.dma_start(xt_sb[:, 2], xt[2 * P : 3 * P, 0:tbs])
                    for ko in (3, 5):
                        nc.scalar.dma_start(
                            xt_sb[:, ko], xt[ko * P : (ko + 1) * P, 0:tbs]
                        )
                    nc.scalar.dma_start(wu_sb[:], wu[:, wc])
                    for ko in (4, 6, 7):
                        nc.gpsimd.dma_start(
                            xt_sb[:, ko], xt[ko * P : (ko + 1) * P, 0:tbs]
                        )
                else:
                    nc.sync.dma_start(wg_sb[:], wg[:, wc])
                    if tb == 0 and wc < 8:
                        # During the DMA ramp the sync queue alone delivers
                        # chunks slower than the half-blocks consume them;
                        # ride the up-weights on the (still idle) gpsimd
                        # queue until the stream has headroom.
                        nc.gpsimd.dma_start(wu_sb[:], wu[:, wc])
                    else:
                        nc.sync.dma_start(wu_sb[:], wu[:, wc])
                if tb == 0 and wc >= 8:
                    # Down weights ride the gpsimd queue after the ramp,
                    # two chunks per wc (needed first at ~75us).
                    for j in (2 * (wc - 8), 2 * (wc - 8) + 1):
                        if j < HO:
                            nc.gpsimd.dma_start(wd_sb[:, j], wd[:, j])
                if tb < NB - 1 and wc == 4:
                    # Prefetch next block's activations on the gpsimd queue.
                    nt0, nts = blocks[tb + 1]
                    xt_sbs[tb + 1] = xpool.tile(
                        [P, KO, nts], BF16, tag="xt", name=f"xt_sb{tb + 1}"
                    )
                    for ko in range(KO):
                        nc.gpsimd.dma_start(
                            xt_sbs[tb + 1][:, ko],
                            xt[ko * P : (ko + 1) * P, nt0 : nt0 + nts],
                        )

                # Gate section.
                gps = [
                    psum.tile([P, 512], F32, tag="acc", bufs=7, name=f"gps{i}")
                    for i in range(nt)
                ]
                if tb == 0 and wc == 0:
                    ko_order = [0, 3, 4, 1, 5, 6, 2, 7]
                else:
                    ko_order = list(range(KO))
                pad_after = {0: 6, 1: 4, 2: 3, 3: 2, 4: 2} \
                    if tb == 0 and wc == 0 else (
                        {0: 2, 1: 2} if tb == 0 and wc == 1 else {})
                for i, ko in enumerate(ko_order):
                    for th in range(nt):
                        nc.tensor.matmul(
                            gps[th][:],
                            wg_sb[:, ko],
                            xt_sb[:, ko, th * 512 : (th + 1) * 512],
                            start=(i == 0),
                            stop=(i == KO - 1),
                        )
                    pad(pad_after.get(i, 0))
                # Up section.
                ups = [
                    psum.tile([P, 512], F32, tag="acc", bufs=7, name=f"ups{i}")
                    for i in range(nt)
                ]
                up_pad = {0: 2, 1: 2, 2: 2} if tb == 0 and wc == 0 else {}
                for i, ko in enumerate(ko_order):
                    for th in range(nt):
                        nc.tensor.matmul(
                            ups[th][:],
                            wu_sb[:, ko],
                            xt_sb[:, ko, th * 512 : (th + 1) * 512],
                            start=(i == 0),
                            stop=(i == KO - 1),
                        )
                    pad(up_pad.get(i, 0))
                # SwiGLU epilogue: silu on scalar engine, mul on vector.
                for th in range(nt):
                    tsl = slice(th * 512, (th + 1) * 512)
                    s = spool.tile([P, 512], BF16, tag="silu")
                    nc.scalar.activation(s[:], gps[th][:], silu_fn)
                    nc.vector.tensor_mul(hid_sb[:, wc, tsl], s[:], ups[th][:])

            # Down projection: wd stationary, hidden moving in [h, t]
            # layout; out lands as [O, T].
            for oc in range(NOC):
                osl = slice(oc * P, (oc + 1) * P)
                last = tb == NB - 1 and oc == NOC - 1
                if last:
                    # Final group: t-tile-outer so each tile's copy+store
                    # overlaps the next tile's matmuls; stores go to two
                    # idle issue queues so the tail is one small transfer.
                    engs = [nc.sync, nc.scalar]
                    for th in range(nt - 1):
                        op_l = psum.tile([P, 512], F32, tag="acc", bufs=7, name="op_l")
                        for ho in range(HO):
                            nc.tensor.matmul(
                                op_l[:],
                                wd_sb[:, ho, osl],
                                hid_sb[:, ho, th * 512 : (th + 1) * 512],
                                start=(ho == 0),
                                stop=(ho == HO - 1),
                            )
                        obl = opool.tile([P, 512], F32, tag="ob", name="obl")
                        nc.vector.tensor_copy(obl[:], op_l[:])
                        engs[th].dma_start(
                            out[osl, t0 + th * 512 : t0 + (th + 1) * 512],
                            obl[:],
                        )
                    # Very last 512 columns as two 256-col accumulations
                    # in separate banks: the first half's copy+store runs
                    # under the second half's matmuls, so the tail after
                    # the final matmul is one small transfer.
                    base = t0 + (nt - 1) * 512
                    for hh in range(2):
                        op_h = psum.tile([P, 256], F32, tag="acc", bufs=7, name="op_h")
                        csl = slice((nt - 1) * 512 + hh * 256,
                                    (nt - 1) * 512 + (hh + 1) * 256)
                        for ho in range(HO):
                            nc.tensor.matmul(
                                op_h[:],
                                wd_sb[:, ho, osl],
                                hid_sb[:, ho, csl],
                                start=(ho == 0),
                                stop=(ho == HO - 1),
                            )
                        obh = opool.tile([P, 256], F32, tag="obh", bufs=2, name="obh")
                        nc.vector.tensor_copy(obh[:], op_h[:])
                        engs[hh].dma_start(
                            out[osl, base + hh * 256 : base + (hh + 1) * 256],
                            obh[:],
                        )
                    continue
                ops = [
                    psum.tile([P, 512], F32, tag="acc", bufs=7, name=f"ops{i}")
                    for i in range(nt)
                ]
                for ho in range(HO):
                    for th in range(nt):
                        nc.tensor.matmul(
                            ops[th][:],
                            wd_sb[:, ho, osl],
                            hid_sb[:, ho, th * 512 : (th + 1) * 512],
                            start=(ho == 0),
                            stop=(ho == HO - 1),
                        )
                ob = opool.tile([P, 1024], F32, tag="obb", name="ob")
                for th in range(nt):
                    nc.vector.tensor_copy(ob[:, th * 512 : (th + 1) * 512], ops[th][:])
                nc.gpsimd.dma_start(out[osl, t0 : t0 + tbs], ob[:, 0:tbs])

    nc.compile()
    return nc


def _get_nc():
    global _CACHED_NC
    if _CACHED_NC is None:
        _CACHED_NC = _build_nc()
    return _CACHED_NC


def _make_in_maps(x, gate_weight, up_weight, down_weight, n):
    in_maps = []
    for g in range(n):
        xtg = np.ascontiguousarray(x[g::n].T.astype(NPBF))
        # [K, H] -> [p, wc, ko, 128] so each weight-chunk DMA descriptor is
        # a contiguous 2KB run per partition.
        wgt = np.ascontiguousarray(
            gate_weight[g].astype(NPBF).reshape(KO, P, NWC, P).transpose(1, 2, 0, 3)
        )
        wut = np.ascontiguousarray(
            up_weight[g].astype(NPBF).reshape(KO, P, NWC, P).transpose(1, 2, 0, 3)
        )
        # [H, O] -> [p, ho, O]
        wdt = np.ascontiguousarray(
            down_weight[g].astype(NPBF).reshape(HO, P, O).transpose(1, 0, 2)
        )
        in_maps.append({"xt": xtg, "wg": wgt, "wu": wut, "wd": wdt})
    return in_maps


def _run_spmd(in_maps, **kwargs):
    nc = _get_nc()
    return run_bass_kernel_spmd(nc, in_maps, core_ids=list(range(N_CORES)), **kwargs)


def kernel(x, gate_weight, up_weight, down_weight, num_groups=8):
    n = int(num_groups)
    x = np.asarray(x, dtype=np.float32)
    gate_weight = np.asarray(gate_weight, dtype=np.float32)
    up_weight = np.asarray(up_weight, dtype=np.float32)
    down_weight = np.asarray(down_weight, dtype=np.float32)

    assert n == N_CORES, f"expected {N_CORES} groups, got {n}"
    assert x.shape == (T * N_CORES, K), x.shape
    assert gate_weight.shape == (n, K, H), gate_weight.shape
    assert up_weight.shape == (n, K, H), up_weight.shape
    assert down_weight.shape == (n, H, O), down_weight.shape

    in_maps = _make_in_maps(x, gate_weight, up_weight, down_weight, n)
    res = _run_spmd(in_maps)

    out = np.empty((x.shape[0], O), dtype=np.float32)
    for g in range(n):
        out[g::n] = res.results[g]["out"].T
    return out
